# revision 1
# baseline (speedup 1.0000x reference)
"""Trainium2 Bass kernel for ConvDownsample2d (FIR blur + 3x3/s2 conv + bias + leaky_relu*sqrt2).

Contract: kernel(**inputs) takes FULL inputs (x[16,512,64,64] f32, weight[512,512,3,3],
bias[512], fir[4,4]) and returns the FULL output [16,512,32,32] f32.

Strategy (hardcoded for this problem size):
  - Data-parallel over batch: 16 images / 8 cores = 2 images per core. No collectives.
  - Host prep: x scaled by fir[0,0] (=1/64) and cast to fp16; weights transposed to
    [cin, 3*3, cout], scaled by W_LRMUL*sqrt2, cast fp16; bias*sqrt2 as [128,4] f32.
  - Device: separable [1,3,3,1] blur on VectorE in fp16 (6 ops/chunk, all operands kept
    4B-aligned via a one-element-shifted second DMA copy of x), then the strided conv as
    accumulated 128x128x512 fp16 matmuls on TensorE (channels on partitions, 9 taps x
    4 cin-chunks into PSUM), epilogue bias+leaky_relu(0.2) on ScalarE, DMA out f32.
"""

import sys

for p in ("/opt/trn_rl_repo", "/opt/pypackages"):
    if p not in sys.path:
        sys.path.insert(0, p)

import numpy as np
from contextlib import ExitStack

from concourse import bass, bacc, mybir, tile
from concourse.bass_utils import run_bass_kernel_spmd

F16 = mybir.dt.float16
F32 = mybir.dt.float32

NCORES = 8
NPC = 2            # images per core
CIN = 512
COUT = 512
H = W = 64
OH = OW = 32
KS = 3
W_LRMUL = 1.0 / np.sqrt(CIN * COUT * KS * KS)
SQRT2 = np.sqrt(2.0)

MT = ML = 4        # top/left margin of padded SBUF tiles
SH = SW = 70       # padded tile extent (4 + 64 + 2)

_CACHE = {}


def _build(reps=1):
    nc = bacc.Bacc("TRN2", target_bir_lowering=False, debug=False, enable_asserts=False)

    x_d = nc.dram_tensor("x", [NPC, CIN, H, W], F16, kind="ExternalInput")
    w_d = nc.dram_tensor("w", [CIN, 9, COUT], F16, kind="ExternalInput")
    b_d = nc.dram_tensor("b", [128, 4], F32, kind="ExternalInput")
    o_d = nc.dram_tensor("out", [NPC, COUT, OH, OW], F32, kind="ExternalOutput")

    with tile.TileContext(nc) as tc, ExitStack() as ctx:
        cpool = ctx.enter_context(tc.tile_pool(name="const", bufs=1))
        bpool = ctx.enter_context(tc.tile_pool(name="blur", bufs=1))
        opool = ctx.enter_context(tc.tile_pool(name="outp", bufs=4))
        ppool = ctx.enter_context(
            tc.tile_pool(name="psum", bufs=1, space=bass.MemorySpace.PSUM)
        )

        # --- constants ---
        w_sb = cpool.tile([128, 4, 9, COUT], F16, name="w_sb")
        for kc in range(4):
            nc.sync.dma_start(out=w_sb[:, kc], in_=w_d[kc * 128:(kc + 1) * 128])
        b_sb = cpool.tile([128, 4], F32, name="b_sb")
        nc.sync.dma_start(out=b_sb[:], in_=b_d[:])

        # --- static double-buffered blur tiles ---
        def pair(tag):
            return [
                bpool.tile([128, SH, SW], F16, tag=f"{tag}{i}", name=f"{tag}{i}")
                for i in range(2)
            ]

        xt, xst, t1t, t2t, zt, yt = (pair(t) for t in ("xt", "xs", "t1", "t2", "zt", "yt"))

        # zero guards once; every later write stays in the interior
        for tl in (*xt, *xst, *zt, *yt):
            nc.scalar.memzero(tl[:])

        AL = mybir.AluOpType

        for n in [i % NPC for i in range(reps * NPC)]:
            psum = [
                [
                    ppool.tile([128, 16, OW], F32, tag=f"ps{mc}{uh}", name=f"ps{mc}{uh}")
                    for uh in range(2)
                ]
                for mc in range(4)
            ]
            for kc in range(4):
                s = (n * 4 + kc) % 2
                x_, xs_, t1, t2, z, y = xt[s], xst[s], t1t[s], t2t[s], zt[s], yt[s]
                cs = x_d[n, kc * 128:(kc + 1) * 128]
                nc.sync.dma_start(out=x_[:, MT:MT + 64, ML:ML + 64], in_=cs)
                # xs[r,c] = x[r,c+1]: derive shifted copy on ScalarE (saves an
                # HBM re-read; keeps all VectorE blur operands 4B-aligned)
                nc.scalar.copy(
                    xs_[:, MT:MT + 64, ML - 1:ML + 63], x_[:, MT:MT + 64, ML:ML + 64]
                )

                # W-blur: z[r,c] = x[c-2] + 3 x[c-1] + 3 x[c] + x[c+1]
                I = (slice(None), slice(MT, MT + 64), slice(ML, ML + 64))
                Im2 = (slice(None), slice(MT, MT + 64), slice(ML - 2, ML + 62))
                nc.vector.tensor_tensor(t1[I], x_[Im2], xs_[I], AL.add)
                nc.vector.tensor_tensor(t2[I], x_[I], xs_[Im2], AL.add)
                nc.vector.scalar_tensor_tensor(z[I], t2[I], 3.0, t1[I], AL.mult, AL.add)

                # H-blur: y[r,c] = z[r-2] + 3 z[r-1] + 3 z[r] + z[r+1]
                def rs(dr):
                    return (slice(None), slice(MT + dr, MT + dr + 64), slice(ML, ML + 64))

                nc.vector.tensor_tensor(t1[I], z[rs(-2)], z[rs(1)], AL.add)
                nc.vector.tensor_tensor(t2[I], z[rs(-1)], z[rs(0)], AL.add)
                nc.vector.scalar_tensor_tensor(y[I], t2[I], 3.0, t1[I], AL.mult, AL.add)

                # conv taps: psum[mc][uh] += w[p,q,kc,mc].T @ y[2u+p-1, 2v+q-1]
                for pq in range(9):
                    p, q = divmod(pq, 3)
                    for mc in range(4):
                        lhsT = w_sb[:, kc, pq, mc * 128:(mc + 1) * 128]
                        for uh in range(2):
                            r0 = MT - 1 + p + 32 * uh
                            c0 = ML - 1 + q
                            rhs = y[:, r0:r0 + 32:2, c0:c0 + 64:2]
                            nc.tensor.matmul(
                                psum[mc][uh][:],
                                lhsT,
                                rhs,
                                start=(kc == 0 and pq == 0),
                                stop=(kc == 3 and pq == 8),
                            )

            # epilogue: out = leaky_relu_0.2(psum + bias)   (sqrt2 folded on host)
            # ScalarE adds bias (exact f32) evacuating PSUM; VectorE does
            # leaky via max(0.2*t, t) in one scalar_tensor_tensor op.
            for mc in range(4):
                for uh in range(2):
                    tb = opool.tile([128, 16, OW], F32, tag="tb", name="tb")
                    nc.scalar.activation(
                        tb[:],
                        psum[mc][uh][:],
                        mybir.ActivationFunctionType.Identity,
                        bias=b_sb[:, mc:mc + 1],
                        scale=1.0,
                    )
                    ob = opool.tile([128, 16, OW], F32, tag="ob", name="ob")
                    # leaky = max(0.2v, v) in one VectorE op
                    nc.vector.scalar_tensor_tensor(
                        ob[:], tb[:], 0.2, tb[:], AL.mult, AL.max
                    )
                    nc.sync.dma_start(
                        out=o_d[n, mc * 128:(mc + 1) * 128, uh * 16:(uh + 1) * 16, :],
                        in_=ob[:],
                    )

    nc.compile()
    return nc


def get_nc(reps=1):
    key = f"nc{reps}"
    if key not in _CACHE:
        _CACHE[key] = _build(reps)
    return _CACHE[key]


def prep_inputs(x, weight, bias, fir):
    """Host-side shard + fold constants. Returns per-core input maps."""
    x = np.asarray(x, dtype=np.float32)
    weight = np.asarray(weight, dtype=np.float32)
    bias = np.asarray(bias, dtype=np.float32)
    fir = np.asarray(fir, dtype=np.float32)

    # normalized separable fir = fir[0,0] * outer([1,3,3,1],[1,3,3,1]);
    # fold fir[0,0] into x, integer taps run on device.
    scale = float(fir[0, 0])
    x_dev = (x * scale).astype(np.float16)

    # w_host[cin, p*3+q, cout] = weight[cout, cin, p, q] * W_LRMUL * sqrt2
    w_host = np.ascontiguousarray(
        (weight.transpose(1, 2, 3, 0) * np.float32(W_LRMUL * SQRT2))
        .reshape(CIN, 9, COUT)
        .astype(np.float16)
    )
    b_host = np.ascontiguousarray(
        (bias * np.float32(SQRT2)).astype(np.float32).reshape(4, 128).T
    )

    in_maps = []
    for c in range(NCORES):
        in_maps.append(
            {
                "x": np.ascontiguousarray(x_dev[c * NPC:(c + 1) * NPC]),
                "w": w_host,
                "b": b_host,
            }
        )
    return in_maps


def run(in_maps, trace=False, **kw):
    nc = get_nc()
    return run_bass_kernel_spmd(nc, in_maps, list(range(NCORES)), trace=trace, **kw)


def kernel(x, weight, bias, fir):
    res = run(prep_inputs(x, weight, bias, fir)).results
    out = np.concatenate([r["out"] for r in res], axis=0)
    return out.astype(np.float32)



# revision 27
# speedup vs baseline: 1.1355x; 1.1355x over previous
"""Trainium2 Bass kernel for ConvDownsample2d (FIR blur + 3x3/s2 conv + bias + leaky_relu*sqrt2).

Contract: kernel(**inputs) takes FULL inputs (x[16,512,64,64] f32, weight[512,512,3,3],
bias[512], fir[4,4]) and returns the FULL output [16,512,32,32] f32.

Strategy (hardcoded for this problem size):
  - Data-parallel over batch: 16 images / 8 cores = 2 images per core. No collectives.
  - Host prep: x scaled by fir[0,0] (=1/64) and cast to fp16; weights transposed to
    [cin, 3*3, cout], scaled by W_LRMUL*sqrt2, cast fp16; bias*sqrt2 as [128,4] f32.
  - Device: separable [1,3,3,1] blur computed as z = (x[-2]+x[+1]) + 3*(x[-1]+x[0])
    per dimension: the two pairwise adds run on VectorE (fp16 2x mode), the fused
    a+3b on GpSimd (Pool) to keep VectorE off the critical path. ScalarE provides
    the one-element-shifted copy of x (keeps DVE operands 4B-aligned).
  - Conv: accumulated 128x128x512 fp16 matmuls on TensorE (channels on partitions,
    9 taps x 4 cin-chunks into PSUM); mc-outer loop order staggers PSUM bank
    completion so the epilogue drains overlap the next output-chunk's matmuls.
  - Epilogue: ScalarE bias-add evacuating PSUM, VectorE leaky via max(0.2v, v),
    DMA out f32 (sqrt2 folded into weights+bias on host).
"""

import sys

for p in ("/opt/trn_rl_repo", "/opt/pypackages"):
    if p not in sys.path:
        sys.path.insert(0, p)

import numpy as np
from contextlib import ExitStack

from concourse import bass, bacc, mybir, tile
from concourse.bass_utils import run_bass_kernel_spmd

F16 = mybir.dt.float16
F32 = mybir.dt.float32

NCORES = 8
NPC = 2            # images per core
N_TOT = 16         # total batch
CIN = 512
COUT = 512
H = W = 64
OH = OW = 32
KS = 3
W_LRMUL = 1.0 / np.sqrt(CIN * COUT * KS * KS)
SQRT2 = np.sqrt(2.0)

MT = 4             # top margin of z/y tiles (rows)
ML = 4             # left margin (cols)
TW = 68            # tile width (cols): interior 4..67, guards below 4
XB = 2             # x prefetch buffers
YB = 3             # y buffers

USE_PRELU = True   # fused ScalarE Prelu epilogue (not implemented in CoreSim;
                   # validated end-to-end on HW via the rel-err check)

_CACHE = {}


def _build(reps=1):
    nc = bacc.Bacc("TRN2", target_bir_lowering=False, debug=False, enable_asserts=False)

    x_d = nc.dram_tensor("x", [NPC, CIN, H, TW], F16, kind="ExternalInput")
    xs_d = nc.dram_tensor("xsh", [NPC, CIN, H, TW], F16, kind="ExternalInput")
    w_d = nc.dram_tensor("w", [CIN, 9, COUT], F16, kind="ExternalInput")
    b_d = nc.dram_tensor("b", [128, 4], F32, kind="ExternalInput")
    o_d = nc.dram_tensor("out", [NPC, COUT, OH, OW], F32, kind="ExternalOutput")

    AL = mybir.AluOpType
    AF = mybir.ActivationFunctionType

    with tile.TileContext(nc) as tc, ExitStack() as ctx:
        cpool = ctx.enter_context(tc.tile_pool(name="const", bufs=1))
        bpool = ctx.enter_context(tc.tile_pool(name="blur", bufs=1))
        opool = ctx.enter_context(tc.tile_pool(name="outp", bufs=8))
        ppool = ctx.enter_context(
            tc.tile_pool(name="psum", bufs=1, space=bass.MemorySpace.PSUM)
        )

        # --- constants (kc=0 weights issued first so the first matmuls and the
        # first blur chain are not stuck behind the full weight transfer) ---
        w_sb = cpool.tile([128, 4, 9, COUT], F16, name="w_sb")
        nc.sync.dma_start(out=w_sb[:, 0], in_=w_d[0:128])
        b_sb = cpool.tile([128, 4], F32, name="b_sb")
        nc.sync.dma_start(out=b_sb[:], in_=b_d[:])

        # --- blur tiles ---
        xt = [bpool.tile([128, 64, TW], F16, name=f"x{i}") for i in range(XB)]
        xst = [bpool.tile([128, 64, TW], F16, name=f"xs{i}") for i in range(2)]
        at = bpool.tile([128, 64, TW], F16, name="at")
        bt = [bpool.tile([128, 64, TW], F16, name=f"bt{i}") for i in range(2)]
        b3 = bpool.tile([128, 64, TW], F16, name="b3")
        zt = bpool.tile([128, 70, TW], F16, name="zt")
        a2 = bpool.tile([128, 64, TW], F16, name="a2")
        b2 = [bpool.tile([128, 64, TW], F16, name=f"b2_{i}") for i in range(2)]
        b23 = bpool.tile([128, 64, TW], F16, name="b23")
        yt = [bpool.tile([128, 70, TW], F16, name=f"y{i}") for i in range(YB)]
        # (buffer counts sized so total SBUF stays under the usable budget)

        # --- zero guards once; every per-chunk write stays in the interior
        # (x/xs guards are baked into the host-padded layout) ---
        nc.vector.memzero(zt[:, 2:4, ML:ML + 64])          # z rows -2, -1
        nc.vector.memzero(zt[:, 68:69, ML:ML + 64])        # z row 64
        for y_ in yt:
            nc.vector.memzero(y_[:, 3:4, 2:TW])            # y row -1 (col 2 unused)
            nc.vector.memzero(y_[:, 4:68, 2:4])            # y col -1 (col 2 unused)

        n_imgs = reps * NPC
        chunks = [(i % NPC, kc) for i in range(n_imgs) for kc in range(4)]

        def dma_x(g):
            n, kc = chunks[g]
            nc.sync.dma_start(
                out=xt[g % XB][:], in_=x_d[n, kc * 128:(kc + 1) * 128]
            )

        def dma_xs(g):
            # host-padded shifted copy xs[c] = x[c+1] (keeps DVE blur operands
            # 4B-aligned without a ScalarE hop); fully contiguous transfer.
            n, kc = chunks[g]
            nc.sync.dma_start(
                out=xst[g % 2][:], in_=xs_d[n, kc * 128:(kc + 1) * 128]
            )

        for g in range(min(XB, len(chunks))):
            dma_x(g)
        for g in range(min(2, len(chunks))):
            dma_xs(g)
        for kc in range(1, 4):
            nc.sync.dma_start(out=w_sb[:, kc], in_=w_d[kc * 128:(kc + 1) * 128])

        for g, (n, kc) in enumerate(chunks):
            x_ = xt[g % XB]
            y_ = yt[g % YB]
            xs = xst[g % 2]
            bt_, b2_ = bt[g % 2], b2[g % 2]

            # W-blur: z[c] = (x[c-2] + x[c+1]) + 3*(x[c-1] + x[c])
            # pairwise adds on VectorE (fp16 2x), the x3 as a ScalarE scaled copy
            nc.vector.tensor_tensor(
                at[:, :, ML:ML + 64], x_[:, :, 2:66], xs[:, :, ML:ML + 64], AL.add
            )
            nc.vector.tensor_tensor(
                bt_[:, :, ML:ML + 64], xs[:, :, 2:66], x_[:, :, ML:ML + 64], AL.add
            )
            if g + XB < len(chunks):
                dma_x(g + XB)
            if g + 2 < len(chunks):
                dma_xs(g + 2)
            nc.scalar.activation(
                b3[:, :, ML:ML + 64], bt_[:, :, ML:ML + 64], AF.Identity, scale=3.0
            )
            nc.vector.tensor_tensor(
                zt[:, MT:MT + 64, ML:ML + 64],
                at[:, :, ML:ML + 64], b3[:, :, ML:ML + 64], AL.add
            )

            # H-blur: y[r] = (z[r-2] + z[r+1]) + 3*(z[r-1] + z[r])
            nc.vector.tensor_tensor(
                a2[:, :, ML:ML + 64],
                zt[:, 2:66, ML:ML + 64], zt[:, 5:69, ML:ML + 64], AL.add
            )
            nc.vector.tensor_tensor(
                b2_[:, :, ML:ML + 64],
                zt[:, 3:67, ML:ML + 64], zt[:, 4:68, ML:ML + 64], AL.add
            )
            nc.scalar.activation(
                b23[:, :, ML:ML + 64], b2_[:, :, ML:ML + 64], AF.Identity, scale=3.0
            )
            nc.vector.tensor_tensor(
                y_[:, MT:MT + 64, ML:ML + 64],
                a2[:, :, ML:ML + 64], b23[:, :, ML:ML + 64], AL.add
            )

            # conv taps: psum[mc][uh] += w[p,q,kc,mc].T @ y[2u+p-1, 2v+q-1]
            if kc == 0:
                psum = [
                    [
                        ppool.tile([128, 16, OW], F32, tag=f"ps{mc}{uh}", name=f"ps{mc}{uh}")
                        for uh in range(2)
                    ]
                    for mc in range(4)
                ]
            for mc in range(4):
                for pq in range(9):
                    p, q = divmod(pq, 3)
                    lhsT = w_sb[:, kc, pq, mc * 128:(mc + 1) * 128]
                    for uh in range(2):
                        r0 = MT - 1 + p + 32 * uh
                        c0 = ML - 1 + q
                        rhs = y_[:, r0:r0 + 32:2, c0:c0 + 63:2]
                        nc.tensor.matmul(
                            psum[mc][uh][:],
                            lhsT,
                            rhs,
                            start=(kc == 0 and pq == 0),
                            stop=(kc == 3 and pq == 8),
                        )
                if kc == 3:
                    # epilogue for this mc: overlaps the next mc's matmuls.
                    # high_priority keeps the PSUM-draining ops at the front of
                    # the ScalarE/VectorE queues so the next image's start=True
                    # matmuls find their banks free.
                    with tc.high_priority():
                        for uh in range(2):
                            if USE_PRELU:
                                ob = opool.tile([128, 16, OW], F32, tag="ob", name="ob")
                                nc.scalar.activation(
                                    ob[:], psum[mc][uh][:], AF.Prelu,
                                    bias=b_sb[:, mc:mc + 1], scale=1.0, alpha=0.2,
                                )
                            else:
                                tb = opool.tile([128, 16, OW], F32, tag="tb", name="tb")
                                nc.scalar.activation(
                                    tb[:], psum[mc][uh][:], AF.Identity,
                                    bias=b_sb[:, mc:mc + 1], scale=1.0,
                                )
                                ob = opool.tile([128, 16, OW], F32, tag="ob", name="ob")
                                # leaky = max(0.2v, v) in one VectorE op
                                nc.vector.scalar_tensor_tensor(
                                    ob[:], tb[:], 0.2, tb[:], AL.mult, AL.max
                                )
                            nc.sync.dma_start(
                                out=o_d[n, mc * 128:(mc + 1) * 128, uh * 16:(uh + 1) * 16, :],
                                in_=ob[:],
                            )

    nc.compile()
    return nc


def get_nc(reps=1):
    key = f"nc{reps}"
    if key not in _CACHE:
        _CACHE[key] = _build(reps)
    return _CACHE[key]


def prep_inputs(x, weight, bias, fir):
    """Host-side shard + fold constants. Returns per-core input maps."""
    x = np.asarray(x, dtype=np.float32)
    weight = np.asarray(weight, dtype=np.float32)
    bias = np.asarray(bias, dtype=np.float32)
    fir = np.asarray(fir, dtype=np.float32)

    # normalized separable fir = fir[0,0] * outer([1,3,3,1],[1,3,3,1]);
    # fold fir[0,0] into x, integer taps run on device.
    scale = float(fir[0, 0])
    x16 = (x * scale).astype(np.float16)
    # pre-padded tile layouts (width TW=68): col c <-> img col c-4 for x,
    # img col c-3 for the one-element-shifted xs; guard zeros baked in.
    x_dev = np.zeros((N_TOT, CIN, H, TW), dtype=np.float16)
    x_dev[:, :, :, 4:68] = x16
    xs_dev = np.zeros((N_TOT, CIN, H, TW), dtype=np.float16)
    xs_dev[:, :, :, 3:67] = x16

    # w_host[cin, p*3+q, cout] = weight[cout, cin, p, q] * W_LRMUL * sqrt2
    w_host = np.ascontiguousarray(
        (weight.transpose(1, 2, 3, 0) * np.float32(W_LRMUL * SQRT2))
        .reshape(CIN, 9, COUT)
        .astype(np.float16)
    )
    b_host = np.ascontiguousarray(
        (bias * np.float32(SQRT2)).astype(np.float32).reshape(4, 128).T
    )

    in_maps = []
    for c in range(NCORES):
        in_maps.append(
            {
                "x": np.ascontiguousarray(x_dev[c * NPC:(c + 1) * NPC]),
                "xsh": np.ascontiguousarray(xs_dev[c * NPC:(c + 1) * NPC]),
                "w": w_host,
                "b": b_host,
            }
        )
    return in_maps


def run(in_maps, trace=False, **kw):
    nc = get_nc()
    return run_bass_kernel_spmd(nc, in_maps, list(range(NCORES)), trace=trace, **kw)


def kernel(x, weight, bias, fir):
    res = run(prep_inputs(x, weight, bias, fir)).results
    out = np.concatenate([r["out"] for r in res], axis=0)
    return out.astype(np.float32)
